# revision 37
# baseline (speedup 1.0000x reference)
"""CLD sde_reverse (Riemann geometry) Trainium2 kernel — v4.

Contract: kernel(u, score_x, t) -> (drift, diffusion), full (unsharded) numpy
arrays, computed on 8 NeuronCores via bass/Tile + run_bass_kernel_spmd.

Sharding: pixels (image rows) are sharded 8 ways; every core sees all 64 batch
elements for its 32 rows. The batch-mean outer product G, the 3x3
inverse/cholesky, and the drift matmuls are all per-pixel, so there are no
cross-core dependencies and no collectives.

Math (per pixel, 3x3):
    G     = alpha * (mean_b s s^T)/norm + (1-alpha)/m_inv * I
    L     = chol(G),  Ginv = adj(G)/det(G)
    A     = beta * L @ Ginv
    drift_x = A @ r
    drift_r = -(beta*L @ x + beta*Gamma * r)     (G @ Ginv = I exactly)
    diffusion_x = 0
    diffusion_r = sqrt(2*beta*Gamma) * (L @ 1)   (batch independent)

Schedule (engine split chosen off the TimelineSim cost model):
  - score streams in (s0/s1 halves first), pair planes s_i*s_j (ACT
    squares, DVE cross mults, the (1,2) mult on Pool) fold 32->16 on DVE
    (fp16 adds run 4x) and the 16 b-slices accumulate on the PE into PSUM;
    G pairs drain on ACT in cholesky dependency order, so the L chain and
    the first drift_r groups start while s2 pairs are still reducing.
  - drift_r needs only L and x: its six groups run first; the adjugate/
    det/inverse runs on Pool meanwhile; A rows assemble on DVE; drift_x
    groups follow.  Products are coefficient-broadcast mults on DVE (three
    late ones on Pool); accumulation is identity matmuls into a [P, 2048]
    PSUM tile per group (drift_r's -beta*Gamma*r term via a BG-scaled
    identity straight from the r tile); one drain per group (ACT mostly,
    sign folded in), then a streaming DMA.
"""

import math

import numpy as np

# ---- model constants (from the reference config) ----
M_INV = 4.0
GAMMA_BIG = 0.04
BETA0 = 4.0
RIEMANN_MIX = 0.5
K_DECAY = 4.5
C = 3
HW = 256
B = 64

N_CORES = 8
ROWS = HW // N_CORES  # 32 rows per core
P = 128               # SBUF partitions
PL = (ROWS * HW) // P  # 64 free pixels per partition

BETA_C = BETA0 * math.sqrt(M_INV)        # 8.0
GAMMA_C = GAMMA_BIG * math.sqrt(M_INV)   # 0.08
BG = BETA_C * GAMMA_C                    # 0.64
BG_SCALE = math.sqrt(2.0 * BETA_C * GAMMA_C)

_PROG_CACHE: dict = {}


def _build_program(ca: float, cid: float, n_reps: int = 1,
                   newton: bool = False, pool_products: int = 3):
    """Build + compile the per-core SPMD bass program.

    ca  = alpha / (B * normalization)   (scale for the raw sum S_ij)
    cid = (1 - alpha) / M_INV           (identity mixture term)
    """
    from contextlib import ExitStack

    import concourse.bacc as bacc
    import concourse.mybir as mybir
    import concourse.tile as tile

    dt = mybir.dt
    op = mybir.AluOpType
    f32 = dt.float32
    f16 = dt.float16
    AF = mybir.ActivationFunctionType

    nc = bacc.Bacc("TRN2", target_bir_lowering=False, debug=False,
                   num_devices=N_CORES)

    s_in = nc.dram_tensor("s_in", [C, P, B, PL], f16,
                          kind="ExternalInput").ap()
    u_in = nc.dram_tensor("u_in", [2 * C, P, B, PL], f16,
                          kind="ExternalInput").ap()
    id_in = nc.dram_tensor("ident", [P, 2 * P], f16,
                           kind="ExternalInput").ap()
    drift_o = nc.dram_tensor("drift", [2 * C, P, B, PL], f16,
                             kind="ExternalOutput").ap()
    dif_o = nc.dram_tensor("dif", [C, P, PL], f32, kind="ExternalOutput").ap()

    HB = B // 2   # batch half

    cid_t = nc.alloc_sbuf_tensor("cid_const", [P, 1], f32)
    nc.gpsimd.memset(cid_t.ap(), float(cid))
    nc.all_engine_barrier()
    cid_ap = cid_t.ap()

    with tile.TileContext(nc) as tc:
      for _rep in range(n_reps):
        with ExitStack() as stack:
            coef = stack.enter_context(tc.tile_pool(name="coef", bufs=1))
            data = stack.enter_context(tc.tile_pool(name="data", bufs=1))
            tmp = stack.enter_context(tc.tile_pool(name="tmp", bufs=2))
            score_pool = stack.enter_context(
                tc.tile_pool(name="score", bufs=1))
            prod_pool = stack.enter_context(tc.tile_pool(name="prod",
                                                         bufs=1))
            gps_pool = stack.enter_context(
                tc.tile_pool(name="gps", bufs=1, space="PSUM"))

            # pin the sqrt-containing ACT table before any Square runs so
            # the table is loaded exactly once
            dum = tmp.tile([P, 1], f32, tag="dum")
            nc.scalar.activation(dum[:], cid_ap, AF.Sqrt)

            ident2 = coef.tile([P, 2 * P], f16, tag="ident2")
            # off the SP ring so the score DMAs dispatch immediately
            nc.scalar.dma_start(out=ident2[:], in_=id_in)
            ident = ident2[:, 0:P]
            bgident = ident2[:, P:2 * P]

            # ---------------- input DMA streams ---------------------------
            # score: s0/s1 halves first (the (0,0),(0,1),(1,1) pairs gate
            # the cholesky start), s2 halves last
            s_t = [score_pool.tile([P, B, PL], f16, tag=f"s{c}",
                                   name=f"s{c}") for c in range(C)]
            for (c, bh) in [(0, 0), (1, 0), (0, 1), (1, 1), (2, 0), (2, 1)]:
                bsl = slice(bh * HB, (bh + 1) * HB)
                nc.sync.dma_start(out=s_t[c][:, bsl, :],
                                  in_=s_in[c, :, bsl, :])
            # u in wave consumption order: drift_r group i of half bh needs
            # x_0..i[bh] for products and r_i[bh] for its BG matmul
            u_t = [data.tile([P, B, PL], f16, tag=f"u{c}",
                             name=f"u{c}") for c in range(2 * C)]
            for bh in range(2):
                bsl = slice(bh * HB, (bh + 1) * HB)
                for c in (0, C + 0, 1, C + 1, 2, C + 2):
                    nc.sync.dma_start(out=u_t[c][:, bsl, :],
                                      in_=u_in[c, :, bsl, :])
            x_t, r_t = u_t[:C], u_t[C:]

            # ---------------- stage A/B machinery --------------------------
            pairs_all = [(0, 0), (0, 1), (1, 1), (0, 2), (1, 2), (2, 2)]
            g = {}
            # all six pair accumulators packed into one PSUM bank
            gps_all = gps_pool.tile([P, 6 * PL], f32, tag="gps_all",
                                    name="gps_all")
            pr_ps = {pp: gps_all[:, k * PL:(k + 1) * PL]
                     for k, pp in enumerate(pairs_all)}

            def pair_half(i, j, bh, pool=False):
                # plane (ACT square / DVE mult / Pool if requested),
                # fold 32->16 on DVE, 16 b-slice matmuls on PE
                bsl = slice(bh * HB, (bh + 1) * HB)
                ph = prod_pool.tile([P, HB, PL], f16, tag="ph",
                                    bufs=6, name="ph")
                if i == j:
                    nc.scalar.activation(ph[:], s_t[i][:, bsl, :],
                                         AF.Square)
                else:
                    eng = nc.gpsimd if pool else nc.vector
                    eng.tensor_tensor(
                        ph[:], s_t[i][:, bsl, :], s_t[j][:, bsl, :],
                        op.mult)
                f1 = prod_pool.tile([P, 16, PL], f16, tag="f1",
                                    bufs=4, name="f1")
                nc.vector.tensor_tensor(f1[:], ph[:, 0:16, :],
                                        ph[:, 16:32, :], op.add)
                for b in range(16):
                    nc.tensor.matmul(
                        pr_ps[(i, j)], ident, f1[:, b, :],
                        start=(bh == 0 and b == 0),
                        stop=(bh == 1 and b == 15),
                        skip_group_check=True)

            def gdrain(i, j):
                gij = coef.tile([P, PL], f32, tag=f"g{i}{j}",
                                name=f"g{i}{j}")
                if i == j:
                    nc.scalar.activation(gij[:], pr_ps[(i, j)],
                                         AF.Identity, bias=cid_ap,
                                         scale=float(ca))
                else:
                    nc.scalar.mul(gij[:], pr_ps[(i, j)], float(ca))
                g[(i, j)] = gij
                g[(j, i)] = gij

            def sqrt_ref(a, tag):
                out = coef.tile([P, PL], f32, tag=tag, name=tag)
                nc.scalar.activation(out[:], a[:], AF.Sqrt)
                if not newton:
                    return out
                r0 = tmp.tile([P, PL], f32, tag="sqr")
                nc.vector.reciprocal(r0[:], out[:])
                ar = tmp.tile([P, PL], f32, tag="sqar")
                nc.vector.tensor_tensor(ar[:], a[:], r0[:], op.mult)
                ref = coef.tile([P, PL], f32, tag=tag + "n", name=tag + "n")
                nc.vector.tensor_tensor(ref[:], out[:], ar[:], op.add)
                out2 = coef.tile([P, PL], f32, tag=tag + "h",
                                 name=tag + "h")
                nc.vector.tensor_scalar(out2[:], ref[:], 0.5, None, op.mult)
                return out2

            def tt(a, b_, o, tag):
                t = coef.tile([P, PL], f32, tag=tag, name=tag)
                nc.vector.tensor_tensor(t[:], a[:], b_[:], o)
                return t

            def ecopy(lt, i, j):
                e = coef.tile([P, 1, PL], f16, tag=f"eL{i}{j}",
                              name=f"eL{i}{j}")
                nc.scalar.mul(e[:, 0, :], lt[:], BETA_C)
                return e[:]

            # ---------------- stage C machinery ----------------------------
            mtmp = stack.enter_context(tc.tile_pool(name="mtmp", bufs=2))
            outs = stack.enter_context(tc.tile_pool(name="outs", bufs=3))
            psum = stack.enter_context(
                tc.tile_pool(name="psum", bufs=2, space="PSUM"))

            def emit(ch_i, bh, coeffs, ins, with_bg, pool_idx=None,
                     drain_dve=False):
                bsl = slice(bh * HB, (bh + 1) * HB)
                n_pe = len(coeffs) + (1 if with_bg else 0)
                prs = []
                for idx, (cc, dd) in enumerate(zip(coeffs, ins)):
                    pr = mtmp.tile([P, HB, PL], f16, tag=f"pr{idx}", bufs=3,
                                   name=f"pr{idx}")
                    bc = cc.broadcast_to([P, HB, PL])
                    eng = nc.gpsimd if idx == pool_idx else nc.vector
                    eng.tensor_tensor(pr[:], dd[:, bsl, :], bc, op.mult)
                    prs.append(pr)
                pss = [psum.tile([P, 1024], f32, tag="ps", bufs=3,
                                 name="ps") for _ in range(2)]
                # term-major, products first (the BG term last: its r half
                # may still be streaming in when the group starts)
                idx = 0
                for pr in prs:
                    rhs = pr[:].rearrange("p b l -> p (b l)")
                    for s2 in range(4):
                        sl = slice(s2 * 512, (s2 + 1) * 512)
                        psl = slice((s2 % 2) * 512, (s2 % 2 + 1) * 512)
                        nc.tensor.matmul(
                            pss[s2 // 2][:, psl], ident, rhs[:, sl],
                            start=(idx == 0), stop=(idx == n_pe - 1),
                            skip_group_check=True)
                    idx += 1
                if with_bg:
                    rfull = r_t[ch_i][:].rearrange("p b l -> p (b l)")
                    for s2 in range(4):
                        sl = slice((s2 % 2) * 512, (s2 % 2 + 1) * 512)
                        gl = slice(bh * 2048 + s2 * 512,
                                   bh * 2048 + (s2 + 1) * 512)
                        nc.tensor.matmul(
                            pss[s2 // 2][:, sl], bgident, rfull[:, gl],
                            start=(idx == 0), stop=True,
                            skip_group_check=True)
                    idx += 1
                sign = -1.0 if with_bg else 1.0
                out_ch = (C + ch_i) if with_bg else ch_i
                o = outs.tile([P, HB, PL], f16, tag="o", name="o")
                for hq, ps in enumerate(pss):
                    src = ps[:].rearrange("p (b l) -> p b l", b=HB // 2)
                    dst = o[:, hq * (HB // 2):(hq + 1) * (HB // 2), :]
                    if drain_dve:
                        nc.vector.tensor_scalar(dst, src, sign, None,
                                                op.mult)
                    else:
                        nc.scalar.mul(dst, src, sign)
                nc.sync.dma_start(out=drift_o[out_ch, :, bsl, :], in_=o[:])

            # ---------------- the schedule ---------------------------------
            # s0/s1 pairs (both halves), then the G drains + cholesky top
            for (i, j) in [(0, 0), (0, 1), (1, 1)]:
                pair_half(i, j, 0)
            for (i, j) in [(0, 0), (0, 1), (1, 1)]:
                pair_half(i, j, 1)
            gdrain(0, 0)
            l00 = sqrt_ref(g[0, 0], "l00")
            gdrain(0, 1)
            gdrain(1, 1)
            eL = {(0, 0): ecopy(l00, 0, 0)}
            il00 = coef.tile([P, PL], f32, tag="il00")
            nc.vector.reciprocal(il00[:], l00[:])
            l10 = tt(g[0, 1], il00, op.mult, "l10")

            # first drift_r group as early as possible
            emit(0, 0, [eL[(0, 0)]], x_t[:1], True)

            t = tt(l10, l10, op.mult, "l10sq")
            dd1 = tt(g[1, 1], t, op.subtract, "dd1")
            l11 = sqrt_ref(dd1, "l11")
            il11 = coef.tile([P, PL], f32, tag="il11")
            nc.vector.reciprocal(il11[:], l11[:])
            eL[(1, 0)] = ecopy(l10, 1, 0)
            eL[(1, 1)] = ecopy(l11, 1, 1)
            emit(1, 0, [eL[(1, 0)], eL[(1, 1)]], x_t[:2], True)

            # s2 pairs; (2,2) squares first on ACT, (1,2)'s h0 on Pool
            pair_half(2, 2, 0)
            pair_half(2, 2, 1)
            pair_half(0, 2, 0)
            pair_half(1, 2, 0, pool=True)
            pair_half(0, 2, 1)
            pair_half(1, 2, 1, pool=True)
            gdrain(0, 2)
            gdrain(1, 2)
            gdrain(2, 2)
            l20 = tt(g[0, 2], il00, op.mult, "l20")
            t = tt(l20, l10, op.mult, "l20l10")
            t = tt(g[1, 2], t, op.subtract, "g12m")
            l21 = tt(t, il11, op.mult, "l21")
            t = tt(l20, l20, op.mult, "l20sq")
            dd2 = tt(g[2, 2], t, op.subtract, "dd2a")
            t = tt(l21, l21, op.mult, "l21sq")
            dd2 = tt(dd2, t, op.subtract, "dd2")
            l22 = sqrt_ref(dd2, "l22")
            L = {(0, 0): l00, (1, 0): l10, (1, 1): l11,
                 (2, 0): l20, (2, 1): l21, (2, 2): l22}
            eL[(2, 0)] = ecopy(l20, 2, 0)
            eL[(2, 1)] = ecopy(l21, 2, 1)
            eL[(2, 2)] = ecopy(l22, 2, 2)
            emit(2, 0, [eL[(2, j)] for j in range(3)], x_t, True)

            # remaining drift_r groups (batch half 1)
            for i in range(3):
                emit(i, 1, [eL[(i, j)] for j in range(i + 1)],
                     x_t[:i + 1], True)

            # adjugate + det on Pool (parallel with the drift_r wave)
            def ptt(a, b_, o, tag):
                t = coef.tile([P, PL], f32, tag=tag, name=tag)
                nc.gpsimd.tensor_tensor(t[:], a[:], b_[:], o)
                return t

            def fmsub(a, b_, c_, d_, tag):
                t1 = tmp.tile([P, PL], f32, tag="fm1")
                nc.gpsimd.tensor_tensor(t1[:], a[:], b_[:], op.mult)
                t2 = tmp.tile([P, PL], f32, tag="fm2")
                nc.gpsimd.tensor_tensor(t2[:], c_[:], d_[:], op.mult)
                t_ = coef.tile([P, PL], f32, tag=tag, name=tag)
                nc.gpsimd.tensor_tensor(t_[:], t1[:], t2[:], op.subtract)
                return t_

            c00 = fmsub(g[1, 1], g[2, 2], g[1, 2], g[1, 2], "c00")
            c01 = fmsub(g[0, 2], g[1, 2], g[0, 1], g[2, 2], "c01")
            c02 = fmsub(g[0, 1], g[1, 2], g[0, 2], g[1, 1], "c02")
            c11 = fmsub(g[0, 0], g[2, 2], g[0, 2], g[0, 2], "c11")
            c12 = fmsub(g[0, 1], g[0, 2], g[0, 0], g[1, 2], "c12")
            c22 = fmsub(g[0, 0], g[1, 1], g[0, 1], g[0, 1], "c22")
            d0 = ptt(g[0, 0], c00, op.mult, "d0")
            d1 = ptt(g[0, 1], c01, op.mult, "d1")
            d2 = ptt(g[0, 2], c02, op.mult, "d2")
            det = ptt(d0, d1, op.add, "deta")
            det = ptt(det, d2, op.add, "det")
            rdet = coef.tile([P, PL], f32, tag="rdet")
            nc.vector.reciprocal(rdet[:], det[:])

            # diffusion rows on Pool while it waits for rdet
            def pts(src, tag):
                d = coef.tile([P, PL], f32, tag=tag, name=tag)
                nc.gpsimd.tensor_scalar(d[:], src[:], BG_SCALE, None,
                                        op.mult)
                return d

            dif0 = pts(L[0, 0], "dif0")
            t = ptt(L[1, 0], L[1, 1], op.add, "difs1")
            dif1 = pts(t, "dif1")
            t = ptt(L[2, 0], L[2, 1], op.add, "difs2a")
            t = ptt(t, L[2, 2], op.add, "difs2")
            dif2 = pts(t, "dif2")
            for i, dtile in enumerate((dif0, dif1, dif2)):
                nc.sync.dma_start(out=dif_o[i], in_=dtile[:])

            IV = [coef.tile([P, 3, PL], f32, tag=f"IV{k}", name=f"IV{k}")
                  for k in range(3)]
            for (i, j), cof in [((0, 0), c00), ((0, 1), c01), ((0, 2), c02),
                                ((1, 1), c11), ((1, 2), c12), ((2, 2), c22)]:
                nc.gpsimd.tensor_tensor(IV[i][:, j, :], cof[:], rdet[:],
                                        op.mult)
                if i != j:
                    nc.gpsimd.tensor_tensor(
                        IV[j][:, i, :], cof[:], rdet[:], op.mult)

            # A-row i = beta * sum_{k<=i} L_ik (broadcast over j) * IV_k
            # (on DVE right before the drift_x wave)
            def lb(i, k):
                return L[(i, k)][:].rearrange(
                    "p l -> p () l").broadcast_to([P, 3, PL])

            eA = {}
            for i in range(3):
                if i == 0:
                    ar = coef.tile([P, 3, PL], f32, tag="AR0", name="AR0")
                    nc.vector.tensor_tensor(ar[:], IV[0][:], lb(0, 0),
                                            op.mult)
                else:
                    acc = tmp.tile([P, 3, PL], f32, tag="Aacc")
                    nc.vector.tensor_tensor(acc[:], IV[0][:], lb(i, 0),
                                            op.mult)
                    for k in range(1, i + 1):
                        pr = tmp.tile([P, 3, PL], f32, tag="Apr")
                        nc.vector.tensor_tensor(pr[:], IV[k][:], lb(i, k),
                                                op.mult)
                        dst = (coef.tile([P, 3, PL], f32, tag=f"AR{i}",
                                         name=f"AR{i}")
                               if k == i else
                               tmp.tile([P, 3, PL], f32, tag="Aacc"))
                        nc.vector.tensor_tensor(dst[:], acc[:], pr[:],
                                                op.add)
                        acc = dst
                    ar = acc
                e = coef.tile([P, 3, PL], f16, tag=f"eAR{i}", name=f"eAR{i}")
                nc.vector.tensor_scalar(e[:], ar[:], BETA_C, None, op.mult)
                for j in range(3):
                    eA[(i, j)] = e[:, j:j + 1, :]

            # drift_x waves; on the last batch-half hand one product per
            # channel to Pool (issued early, consumed late) and drain the
            # last two groups on the by-then-idle DVE
            for bh in range(2):
                for i in range(3):
                    pidx = 0 if (bh == 1 and i < pool_products) else None
                    emit(i, bh, [eA[(i, j)] for j in range(3)], r_t, False,
                         pool_idx=pidx, drain_dve=(bh == 1 and i == 2))

    nc.compile()
    return nc


def _host_reference(u, score_x, t):
    """Pure-numpy fallback (exact reference math); used only when t[0]==1.0
    (the stateful normalization branch, never hit with uniform t)."""
    x, r = u[:, :C], u[:, C:]
    s = np.transpose(score_x, (0, 2, 3, 1)).astype(np.float32)
    G = np.einsum("bhwi,bhwj->hwij", s, s) / np.float32(score_x.shape[0])
    t0 = t[0]
    diag_mean = np.mean(np.trace(G, axis1=-2, axis2=-1)) / C
    normalization = np.where(t0 == 1.0, diag_mean * M_INV, 1.0)
    G = G / normalization
    G_id = (1.0 / M_INV) * np.eye(C, dtype=G.dtype)
    alpha = RIEMANN_MIX * np.exp(-K_DECAY * (1.0 - t0))
    G = alpha * G + (1.0 - alpha) * G_id
    G_inv = np.linalg.inv(G).astype(np.float32)
    G_sqrt = np.linalg.cholesky(G).astype(np.float32)

    def mm(Amat, Bf):
        return np.einsum("hwij,bjhw->bihw", Amat, Bf).astype(np.float32)

    hr = mm(G_inv, r)
    drift_x = BETA_C * mm(G_sqrt, hr)
    drift_r = (-BETA_C * mm(G_sqrt, x) - BETA_C * GAMMA_C * mm(G, hr))
    diffusion_x = np.zeros_like(x)
    diffusion_r = BG_SCALE * mm(G_sqrt, np.ones_like(r))
    drift = np.concatenate((drift_x, drift_r), axis=1)
    diffusion = np.concatenate((diffusion_x, diffusion_r), axis=1)
    return drift, diffusion


def _make_in_maps(u, score_x):
    ident2 = np.zeros((P, 2 * P), dtype=np.float16)
    ident2[:, 0:P] = np.eye(P, dtype=np.float16)
    ident2[:, P:2 * P] = np.float16(BG) * np.eye(P, dtype=np.float16)
    in_maps = []
    for k in range(N_CORES):
        rows = slice(k * ROWS, (k + 1) * ROWS)
        s_np = (score_x[:, :, rows, :]
                .reshape(B, C, P, PL).transpose(1, 2, 0, 3)
                .astype(np.float16))
        u_np = (u[:, :, rows, :]
                .reshape(B, 2 * C, P, PL).transpose(1, 2, 0, 3)
                .astype(np.float16))
        in_maps.append({
            "s_in": np.ascontiguousarray(s_np),
            "u_in": np.ascontiguousarray(u_np),
            "ident": ident2,
        })
    return in_maps


def kernel(u, score_x, t):
    from concourse.bass_utils import run_bass_kernel_spmd

    u = np.ascontiguousarray(np.asarray(u, dtype=np.float32))
    score_x = np.ascontiguousarray(np.asarray(score_x, dtype=np.float32))
    t = np.asarray(t, dtype=np.float32)

    t0 = float(t[0])
    if t0 == 1.0:
        return _host_reference(u, score_x, t)

    alpha = RIEMANN_MIX * math.exp(-K_DECAY * (1.0 - t0))
    ca = alpha / B          # normalization == 1.0 on this branch
    cid = (1.0 - alpha) / M_INV

    key = (round(ca, 12), round(cid, 12))
    nc = _PROG_CACHE.get(key)
    if nc is None:
        nc = _build_program(ca, cid)
        _PROG_CACHE[key] = nc

    in_maps = _make_in_maps(u, score_x)
    res = run_bass_kernel_spmd(nc, in_maps, list(range(N_CORES)))

    drift = np.empty((B, 2 * C, HW, HW), dtype=np.float32)
    diffusion = np.zeros((B, 2 * C, HW, HW), dtype=np.float32)
    for k in range(N_CORES):
        rows = slice(k * ROWS, (k + 1) * ROWS)
        dk = res.results[k]["drift"].astype(np.float32)     # [6, P, B, PL]
        drift[:, :, rows, :] = dk.transpose(2, 0, 1, 3).reshape(
            B, 2 * C, ROWS, HW)
        difk = res.results[k]["dif"].reshape(C, ROWS, HW)   # [3, P, PL]
        diffusion[:, C:, rows, :] = difk[None, :, :, :]
    return drift, diffusion


# revision 44
# speedup vs baseline: 1.4434x; 1.4434x over previous
"""CLD sde_reverse (Riemann geometry) Trainium2 kernel — v4.

Contract: kernel(u, score_x, t) -> (drift, diffusion), full (unsharded) numpy
arrays, computed on 8 NeuronCores via bass/Tile + run_bass_kernel_spmd.

Sharding: pixels (image rows) are sharded 8 ways; every core sees all 64 batch
elements for its 32 rows. The batch-mean outer product G, the 3x3
inverse/cholesky, and the drift matmuls are all per-pixel, so there are no
cross-core dependencies and no collectives.

Math (per pixel, 3x3):
    G     = alpha * (mean_b s s^T)/norm + (1-alpha)/m_inv * I
    L     = chol(G),  Ginv = adj(G)/det(G)
    A     = beta * L @ Ginv
    drift_x = A @ r
    drift_r = -(beta*L @ x + beta*Gamma * r)     (G @ Ginv = I exactly)
    diffusion_x = 0
    diffusion_r = sqrt(2*beta*Gamma) * (L @ 1)   (batch independent)

Schedule (engine split chosen off the TimelineSim cost model):
  - score streams in (s0/s1 halves first), pair planes s_i*s_j (ACT
    squares, DVE cross mults, the (1,2) mult on Pool) fold 32->16 on DVE
    (fp16 adds run 4x) and the 16 b-slices accumulate on the PE into PSUM;
    G pairs drain on ACT in cholesky dependency order, so the L chain and
    the first drift_r groups start while s2 pairs are still reducing.
  - drift_r needs only L and x: its six groups run first; the adjugate/
    det/inverse runs on Pool meanwhile; A rows assemble on DVE; drift_x
    groups follow.  Products are coefficient-broadcast mults on DVE (three
    late ones on Pool); accumulation is identity matmuls into a [P, 2048]
    PSUM tile per group (drift_r's -beta*Gamma*r term via a BG-scaled
    identity straight from the r tile); one drain per group (ACT mostly,
    sign folded in), then a streaming DMA.
"""

import math

import numpy as np

# ---- model constants (from the reference config) ----
M_INV = 4.0
GAMMA_BIG = 0.04
BETA0 = 4.0
RIEMANN_MIX = 0.5
K_DECAY = 4.5
C = 3
HW = 256
B = 64

N_CORES = 8
ROWS = HW // N_CORES  # 32 rows per core
P = 128               # SBUF partitions
PL = (ROWS * HW) // P  # 64 free pixels per partition

BETA_C = BETA0 * math.sqrt(M_INV)        # 8.0
GAMMA_C = GAMMA_BIG * math.sqrt(M_INV)   # 0.08
BG = BETA_C * GAMMA_C                    # 0.64
BG_SCALE = math.sqrt(2.0 * BETA_C * GAMMA_C)

_PROG_CACHE: dict = {}


def _build_program(ca: float, cid: float, n_reps: int = 1,
                   newton: bool = True, pool_products: int = 3,
                   serialize: bool = False):
    """Build + compile the per-core SPMD bass program.

    ca  = alpha / (B * normalization)   (scale for the raw sum S_ij)
    cid = (1 - alpha) / M_INV           (identity mixture term)
    """
    from contextlib import ExitStack

    import concourse.bacc as bacc
    import concourse.mybir as mybir
    import concourse.tile as tile

    dt = mybir.dt
    op = mybir.AluOpType
    f32 = dt.float32
    f16 = dt.float16
    AF = mybir.ActivationFunctionType

    nc = bacc.Bacc("TRN2", target_bir_lowering=False, debug=False,
                   num_devices=N_CORES)

    s_in = nc.dram_tensor("s_in", [C, P, B, PL], f16,
                          kind="ExternalInput").ap()
    u_in = nc.dram_tensor("u_in", [2 * C, P, B, PL], f16,
                          kind="ExternalInput").ap()
    id_in = nc.dram_tensor("ident", [P, 2 * P], f16,
                           kind="ExternalInput").ap()
    drift_o = nc.dram_tensor("drift", [2 * C, P, B, PL], f16,
                             kind="ExternalOutput").ap()
    dif_o = nc.dram_tensor("dif", [C, P, PL], f32, kind="ExternalOutput").ap()

    HB = B // 2   # batch half

    with tile.TileContext(nc) as tc:
      for _rep in range(n_reps):
        with ExitStack() as stack:
            coef = stack.enter_context(tc.tile_pool(name="coef", bufs=1))
            data = stack.enter_context(tc.tile_pool(name="data", bufs=1))
            tmp = stack.enter_context(tc.tile_pool(name="tmp", bufs=2))
            score_pool = stack.enter_context(
                tc.tile_pool(name="score", bufs=1))
            prod_pool = stack.enter_context(tc.tile_pool(name="prod",
                                                         bufs=1))
            gps_pool = stack.enter_context(
                tc.tile_pool(name="gps", bufs=1, space="PSUM"))

            # cid constant as a tracked tile (bias operand of the G drains)
            cid_tile = coef.tile([P, 1], f32, tag="cid")
            nc.gpsimd.memset(cid_tile[:], float(cid))
            cid_ap = cid_tile[:]

            # pin the sqrt-containing ACT table before any Square runs so
            # the table is loaded exactly once
            dum = tmp.tile([P, 1], f32, tag="dum")
            nc.scalar.activation(dum[:], cid_ap, AF.Sqrt)

            ident2 = coef.tile([P, 2 * P], f16, tag="ident2")
            # off the SP ring so the score DMAs dispatch immediately
            nc.scalar.dma_start(out=ident2[:], in_=id_in)
            ident = ident2[:, 0:P]
            bgident = ident2[:, P:2 * P]

            # ---------------- input DMA streams ---------------------------
            # score: s0/s1 halves first (the (0,0),(0,1),(1,1) pairs gate
            # the cholesky start), s2 halves last
            s_t = [score_pool.tile([P, B, PL], f16, tag=f"s{c}",
                                   name=f"s{c}") for c in range(C)]
            for (c, bh) in [(0, 0), (1, 0), (0, 1), (1, 1), (2, 0), (2, 1)]:
                bsl = slice(bh * HB, (bh + 1) * HB)
                nc.sync.dma_start(out=s_t[c][:, bsl, :],
                                  in_=s_in[c, :, bsl, :])
            # u in wave consumption order: drift_r group i of half bh needs
            # x_0..i[bh] for products and r_i[bh] for its BG matmul
            u_t = [data.tile([P, B, PL], f16, tag=f"u{c}",
                             name=f"u{c}") for c in range(2 * C)]
            for bh in range(2):
                bsl = slice(bh * HB, (bh + 1) * HB)
                for c in (0, C + 0, 1, C + 1, 2, C + 2):
                    nc.sync.dma_start(out=u_t[c][:, bsl, :],
                                      in_=u_in[c, :, bsl, :])
            x_t, r_t = u_t[:C], u_t[C:]

            # ---------------- stage A/B machinery --------------------------
            pairs_all = [(0, 0), (0, 1), (1, 1), (0, 2), (1, 2), (2, 2)]
            g = {}
            # all six pair accumulators packed into one PSUM bank
            gps_all = gps_pool.tile([P, 6 * PL], f32, tag="gps_all",
                                    name="gps_all")
            pr_ps = {pp: gps_all[:, k * PL:(k + 1) * PL]
                     for k, pp in enumerate(pairs_all)}

            def pair_half(i, j, bh, pool=False):
                # plane (ACT square / DVE mult / Pool if requested),
                # fold 32->16 on DVE, 16 b-slice matmuls on PE
                bsl = slice(bh * HB, (bh + 1) * HB)
                ph = prod_pool.tile([P, HB, PL], f16, tag="ph",
                                    bufs=6, name="ph")
                if i == j:
                    nc.scalar.activation(ph[:], s_t[i][:, bsl, :],
                                         AF.Square)
                else:
                    eng = nc.gpsimd if pool else nc.vector
                    eng.tensor_tensor(
                        ph[:], s_t[i][:, bsl, :], s_t[j][:, bsl, :],
                        op.mult)
                f1 = prod_pool.tile([P, 16, PL], f16, tag="f1",
                                    bufs=4, name="f1")
                nc.vector.tensor_tensor(f1[:], ph[:, 0:16, :],
                                        ph[:, 16:32, :], op.add)
                for b in range(16):
                    nc.tensor.matmul(
                        pr_ps[(i, j)], ident, f1[:, b, :],
                        start=(bh == 0 and b == 0),
                        stop=(bh == 1 and b == 15),
                        skip_group_check=True)

            def gdrain(i, j):
                gij = coef.tile([P, PL], f32, tag=f"g{i}{j}",
                                name=f"g{i}{j}")
                if i == j:
                    nc.scalar.activation(gij[:], pr_ps[(i, j)],
                                         AF.Identity, bias=cid_ap,
                                         scale=float(ca))
                else:
                    nc.scalar.mul(gij[:], pr_ps[(i, j)], float(ca))
                g[(i, j)] = gij
                g[(j, i)] = gij

            def sqrt_ref(a, tag):
                out = coef.tile([P, PL], f32, tag=tag, name=tag)
                nc.scalar.activation(out[:], a[:], AF.Sqrt)
                if not newton:
                    return out
                r0 = tmp.tile([P, PL], f32, tag="sqr")
                nc.vector.reciprocal(r0[:], out[:])
                ar = tmp.tile([P, PL], f32, tag="sqar")
                nc.vector.tensor_tensor(ar[:], a[:], r0[:], op.mult)
                ref = coef.tile([P, PL], f32, tag=tag + "n", name=tag + "n")
                nc.vector.tensor_tensor(ref[:], out[:], ar[:], op.add)
                out2 = coef.tile([P, PL], f32, tag=tag + "h",
                                 name=tag + "h")
                nc.vector.tensor_scalar(out2[:], ref[:], 0.5, None, op.mult)
                return out2

            def tt(a, b_, o, tag):
                t = coef.tile([P, PL], f32, tag=tag, name=tag)
                nc.vector.tensor_tensor(t[:], a[:], b_[:], o)
                return t

            def ecopy(lt, i, j):
                e = coef.tile([P, 1, PL], f16, tag=f"eL{i}{j}",
                              name=f"eL{i}{j}")
                nc.scalar.mul(e[:, 0, :], lt[:], BETA_C)
                return e[:]

            # ---------------- stage C machinery ----------------------------
            mtmp = stack.enter_context(tc.tile_pool(name="mtmp", bufs=2))
            outs = stack.enter_context(tc.tile_pool(name="outs", bufs=3))
            psum = stack.enter_context(
                tc.tile_pool(name="psum", bufs=2, space="PSUM"))

            def emit(ch_i, bh, coeffs, ins, with_bg, pool_idx=None,
                     drain_dve=False):
                bsl = slice(bh * HB, (bh + 1) * HB)
                n_pe = len(coeffs) + (1 if with_bg else 0)
                prs = []
                for idx, (cc, dd) in enumerate(zip(coeffs, ins)):
                    pr = mtmp.tile([P, HB, PL], f16, tag=f"pr{idx}", bufs=3,
                                   name=f"pr{idx}")
                    bc = cc.broadcast_to([P, HB, PL])
                    eng = nc.gpsimd if idx == pool_idx else nc.vector
                    eng.tensor_tensor(pr[:], dd[:, bsl, :], bc, op.mult)
                    prs.append(pr)
                pss = [psum.tile([P, 1024], f32, tag="ps", bufs=3,
                                 name="ps") for _ in range(2)]
                # term-major, products first (the BG term last: its r half
                # may still be streaming in when the group starts)
                idx = 0
                for pr in prs:
                    rhs = pr[:].rearrange("p b l -> p (b l)")
                    for s2 in range(4):
                        sl = slice(s2 * 512, (s2 + 1) * 512)
                        psl = slice((s2 % 2) * 512, (s2 % 2 + 1) * 512)
                        nc.tensor.matmul(
                            pss[s2 // 2][:, psl], ident, rhs[:, sl],
                            start=(idx == 0), stop=(idx == n_pe - 1),
                            skip_group_check=True)
                    idx += 1
                if with_bg:
                    rfull = r_t[ch_i][:].rearrange("p b l -> p (b l)")
                    for s2 in range(4):
                        sl = slice((s2 % 2) * 512, (s2 % 2 + 1) * 512)
                        gl = slice(bh * 2048 + s2 * 512,
                                   bh * 2048 + (s2 + 1) * 512)
                        nc.tensor.matmul(
                            pss[s2 // 2][:, sl], bgident, rfull[:, gl],
                            start=(idx == 0), stop=True,
                            skip_group_check=True)
                    idx += 1
                sign = -1.0 if with_bg else 1.0
                out_ch = (C + ch_i) if with_bg else ch_i
                o = outs.tile([P, HB, PL], f16, tag="o", name="o")
                for hq, ps in enumerate(pss):
                    src = ps[:].rearrange("p (b l) -> p b l", b=HB // 2)
                    dst = o[:, hq * (HB // 2):(hq + 1) * (HB // 2), :]
                    if drain_dve:
                        nc.vector.tensor_scalar(dst, src, sign, None,
                                                op.mult)
                    else:
                        nc.scalar.mul(dst, src, sign)
                nc.sync.dma_start(out=drift_o[out_ch, :, bsl, :], in_=o[:])
                return o

            # ---------------- the schedule ---------------------------------
            # s0/s1 pairs (both halves), then the G drains + cholesky top
            for (i, j) in [(0, 0), (0, 1), (1, 1)]:
                pair_half(i, j, 0)
            for (i, j) in [(0, 0), (0, 1), (1, 1)]:
                pair_half(i, j, 1)
            gdrain(0, 0)
            l00 = sqrt_ref(g[0, 0], "l00")
            gdrain(0, 1)
            gdrain(1, 1)
            eL = {(0, 0): ecopy(l00, 0, 0)}
            il00 = coef.tile([P, PL], f32, tag="il00")
            nc.vector.reciprocal(il00[:], l00[:])
            l10 = tt(g[0, 1], il00, op.mult, "l10")

            # first drift_r group as early as possible
            emit(0, 0, [eL[(0, 0)]], x_t[:1], True)

            t = tt(l10, l10, op.mult, "l10sq")
            dd1 = tt(g[1, 1], t, op.subtract, "dd1")
            l11 = sqrt_ref(dd1, "l11")
            il11 = coef.tile([P, PL], f32, tag="il11")
            nc.vector.reciprocal(il11[:], l11[:])
            eL[(1, 0)] = ecopy(l10, 1, 0)
            eL[(1, 1)] = ecopy(l11, 1, 1)
            emit(1, 0, [eL[(1, 0)], eL[(1, 1)]], x_t[:2], True)

            # s2 pairs; (2,2) squares first on ACT, (1,2)'s h0 on Pool
            pair_half(2, 2, 0)
            pair_half(2, 2, 1)
            pair_half(0, 2, 0)
            pair_half(1, 2, 0, pool=True)
            pair_half(0, 2, 1)
            pair_half(1, 2, 1, pool=True)
            gdrain(0, 2)
            gdrain(1, 2)
            gdrain(2, 2)
            l20 = tt(g[0, 2], il00, op.mult, "l20")
            t = tt(l20, l10, op.mult, "l20l10")
            t = tt(g[1, 2], t, op.subtract, "g12m")
            l21 = tt(t, il11, op.mult, "l21")
            t = tt(l20, l20, op.mult, "l20sq")
            dd2 = tt(g[2, 2], t, op.subtract, "dd2a")
            t = tt(l21, l21, op.mult, "l21sq")
            dd2 = tt(dd2, t, op.subtract, "dd2")
            l22 = sqrt_ref(dd2, "l22")
            L = {(0, 0): l00, (1, 0): l10, (1, 1): l11,
                 (2, 0): l20, (2, 1): l21, (2, 2): l22}
            eL[(2, 0)] = ecopy(l20, 2, 0)
            eL[(2, 1)] = ecopy(l21, 2, 1)
            eL[(2, 2)] = ecopy(l22, 2, 2)
            emit(2, 0, [eL[(2, j)] for j in range(3)], x_t, True)

            # remaining drift_r groups (batch half 1)
            for i in range(3):
                emit(i, 1, [eL[(i, j)] for j in range(i + 1)],
                     x_t[:i + 1], True)

            # adjugate + det on Pool (parallel with the drift_r wave)
            def ptt(a, b_, o, tag):
                t = coef.tile([P, PL], f32, tag=tag, name=tag)
                nc.gpsimd.tensor_tensor(t[:], a[:], b_[:], o)
                return t

            def fmsub(a, b_, c_, d_, tag):
                t1 = tmp.tile([P, PL], f32, tag="fm1")
                nc.gpsimd.tensor_tensor(t1[:], a[:], b_[:], op.mult)
                t2 = tmp.tile([P, PL], f32, tag="fm2")
                nc.gpsimd.tensor_tensor(t2[:], c_[:], d_[:], op.mult)
                t_ = coef.tile([P, PL], f32, tag=tag, name=tag)
                nc.gpsimd.tensor_tensor(t_[:], t1[:], t2[:], op.subtract)
                return t_

            c00 = fmsub(g[1, 1], g[2, 2], g[1, 2], g[1, 2], "c00")
            c01 = fmsub(g[0, 2], g[1, 2], g[0, 1], g[2, 2], "c01")
            c02 = fmsub(g[0, 1], g[1, 2], g[0, 2], g[1, 1], "c02")
            c11 = fmsub(g[0, 0], g[2, 2], g[0, 2], g[0, 2], "c11")
            c12 = fmsub(g[0, 1], g[0, 2], g[0, 0], g[1, 2], "c12")
            c22 = fmsub(g[0, 0], g[1, 1], g[0, 1], g[0, 1], "c22")
            d0 = ptt(g[0, 0], c00, op.mult, "d0")
            d1 = ptt(g[0, 1], c01, op.mult, "d1")
            d2 = ptt(g[0, 2], c02, op.mult, "d2")
            det = ptt(d0, d1, op.add, "deta")
            det = ptt(det, d2, op.add, "det")
            rdet = coef.tile([P, PL], f32, tag="rdet")
            nc.vector.reciprocal(rdet[:], det[:])

            # diffusion rows on Pool while it waits for rdet
            def pts(src, tag):
                d = coef.tile([P, PL], f32, tag=tag, name=tag)
                nc.gpsimd.tensor_scalar(d[:], src[:], BG_SCALE, None,
                                        op.mult)
                return d

            dif0 = pts(L[0, 0], "dif0")
            t = ptt(L[1, 0], L[1, 1], op.add, "difs1")
            dif1 = pts(t, "dif1")
            t = ptt(L[2, 0], L[2, 1], op.add, "difs2a")
            t = ptt(t, L[2, 2], op.add, "difs2")
            dif2 = pts(t, "dif2")
            for i, dtile in enumerate((dif0, dif1, dif2)):
                nc.sync.dma_start(out=dif_o[i], in_=dtile[:])

            IV = [coef.tile([P, 3, PL], f32, tag=f"IV{k}", name=f"IV{k}")
                  for k in range(3)]
            for (i, j), cof in [((0, 0), c00), ((0, 1), c01), ((0, 2), c02),
                                ((1, 1), c11), ((1, 2), c12), ((2, 2), c22)]:
                nc.gpsimd.tensor_tensor(IV[i][:, j, :], cof[:], rdet[:],
                                        op.mult)
                if i != j:
                    nc.gpsimd.tensor_tensor(
                        IV[j][:, i, :], cof[:], rdet[:], op.mult)

            # A-row i = beta * sum_{k<=i} L_ik (broadcast over j) * IV_k
            # (on DVE right before the drift_x wave)
            def lb(i, k):
                return L[(i, k)][:].rearrange(
                    "p l -> p () l").broadcast_to([P, 3, PL])

            eA = {}
            for i in range(3):
                if i == 0:
                    ar = coef.tile([P, 3, PL], f32, tag="AR0", name="AR0")
                    nc.vector.tensor_tensor(ar[:], IV[0][:], lb(0, 0),
                                            op.mult)
                else:
                    acc = tmp.tile([P, 3, PL], f32, tag="Aacc")
                    nc.vector.tensor_tensor(acc[:], IV[0][:], lb(i, 0),
                                            op.mult)
                    for k in range(1, i + 1):
                        pr = tmp.tile([P, 3, PL], f32, tag="Apr")
                        nc.vector.tensor_tensor(pr[:], IV[k][:], lb(i, k),
                                                op.mult)
                        dst = (coef.tile([P, 3, PL], f32, tag=f"AR{i}",
                                         name=f"AR{i}")
                               if k == i else
                               tmp.tile([P, 3, PL], f32, tag="Aacc"))
                        nc.vector.tensor_tensor(dst[:], acc[:], pr[:],
                                                op.add)
                        acc = dst
                    ar = acc
                e = coef.tile([P, 3, PL], f16, tag=f"eAR{i}", name=f"eAR{i}")
                nc.vector.tensor_scalar(e[:], ar[:], BETA_C, None, op.mult)
                for j in range(3):
                    eA[(i, j)] = e[:, j:j + 1, :]

            # drift_x waves; on the last batch-half hand one product per
            # channel to Pool (issued early, consumed late) and drain the
            # last group on the by-then-idle DVE
            o_last = None
            for bh in range(2):
                for i in range(3):
                    pidx = 0 if (bh == 1 and i < pool_products) else None
                    o_last = emit(i, bh, [eA[(i, j)] for j in range(3)],
                                  r_t, False, pool_idx=pidx,
                                  drain_dve=(bh == 1 and i == 2))

            if serialize and _rep < n_reps - 1:
                # measurement mode: chain rep boundaries so the per-rep
                # slope equals the single-pass span (next rep's first
                # input DMAs wait for this rep's last drain)
                tok = tmp.tile([P, 1], f16, tag="tok")
                nc.vector.tensor_tensor(tok[:], s_t[0][:, 0, 0:1],
                                        o_last[:, 0, 0:1], op.add)
                tok2 = tmp.tile([P, 1], f16, tag="tok2")
                nc.vector.tensor_tensor(tok2[:], u_t[0][:, 0, 0:1],
                                        o_last[:, 0, 0:1], op.add)

    nc.compile()
    return nc


def _host_reference(u, score_x, t):
    """Pure-numpy fallback (exact reference math); used only when t[0]==1.0
    (the stateful normalization branch, never hit with uniform t)."""
    x, r = u[:, :C], u[:, C:]
    s = np.transpose(score_x, (0, 2, 3, 1)).astype(np.float32)
    G = np.einsum("bhwi,bhwj->hwij", s, s) / np.float32(score_x.shape[0])
    t0 = t[0]
    diag_mean = np.mean(np.trace(G, axis1=-2, axis2=-1)) / C
    normalization = np.where(t0 == 1.0, diag_mean * M_INV, 1.0)
    G = G / normalization
    G_id = (1.0 / M_INV) * np.eye(C, dtype=G.dtype)
    alpha = RIEMANN_MIX * np.exp(-K_DECAY * (1.0 - t0))
    G = alpha * G + (1.0 - alpha) * G_id
    G_inv = np.linalg.inv(G).astype(np.float32)
    G_sqrt = np.linalg.cholesky(G).astype(np.float32)

    def mm(Amat, Bf):
        return np.einsum("hwij,bjhw->bihw", Amat, Bf).astype(np.float32)

    hr = mm(G_inv, r)
    drift_x = BETA_C * mm(G_sqrt, hr)
    drift_r = (-BETA_C * mm(G_sqrt, x) - BETA_C * GAMMA_C * mm(G, hr))
    diffusion_x = np.zeros_like(x)
    diffusion_r = BG_SCALE * mm(G_sqrt, np.ones_like(r))
    drift = np.concatenate((drift_x, drift_r), axis=1)
    diffusion = np.concatenate((diffusion_x, diffusion_r), axis=1)
    return drift, diffusion


def _make_in_maps(u, score_x):
    ident2 = np.zeros((P, 2 * P), dtype=np.float16)
    ident2[:, 0:P] = np.eye(P, dtype=np.float16)
    ident2[:, P:2 * P] = np.float16(BG) * np.eye(P, dtype=np.float16)
    in_maps = []
    for k in range(N_CORES):
        rows = slice(k * ROWS, (k + 1) * ROWS)
        s_np = (score_x[:, :, rows, :]
                .reshape(B, C, P, PL).transpose(1, 2, 0, 3)
                .astype(np.float16))
        u_np = (u[:, :, rows, :]
                .reshape(B, 2 * C, P, PL).transpose(1, 2, 0, 3)
                .astype(np.float16))
        in_maps.append({
            "s_in": np.ascontiguousarray(s_np),
            "u_in": np.ascontiguousarray(u_np),
            "ident": ident2,
        })
    return in_maps


def kernel(u, score_x, t):
    from concourse.bass_utils import run_bass_kernel_spmd

    u = np.ascontiguousarray(np.asarray(u, dtype=np.float32))
    score_x = np.ascontiguousarray(np.asarray(score_x, dtype=np.float32))
    t = np.asarray(t, dtype=np.float32)

    t0 = float(t[0])
    if t0 == 1.0:
        return _host_reference(u, score_x, t)

    alpha = RIEMANN_MIX * math.exp(-K_DECAY * (1.0 - t0))
    ca = alpha / B          # normalization == 1.0 on this branch
    cid = (1.0 - alpha) / M_INV

    key = (round(ca, 12), round(cid, 12))
    nc = _PROG_CACHE.get(key)
    if nc is None:
        nc = _build_program(ca, cid)
        _PROG_CACHE[key] = nc

    in_maps = _make_in_maps(u, score_x)
    res = run_bass_kernel_spmd(nc, in_maps, list(range(N_CORES)))

    drift = np.empty((B, 2 * C, HW, HW), dtype=np.float32)
    diffusion = np.zeros((B, 2 * C, HW, HW), dtype=np.float32)
    for k in range(N_CORES):
        rows = slice(k * ROWS, (k + 1) * ROWS)
        dk = res.results[k]["drift"].astype(np.float32)     # [6, P, B, PL]
        drift[:, :, rows, :] = dk.transpose(2, 0, 1, 3).reshape(
            B, 2 * C, ROWS, HW)
        difk = res.results[k]["dif"].reshape(C, ROWS, HW)   # [3, P, PL]
        diffusion[:, C:, rows, :] = difk[None, :, :, :]
    return drift, diffusion
